# revision 1
# baseline (speedup 1.0000x reference)
"""Trainium2 Bass kernel for nn_DQA_89077621719347 (dense_cnn, 8 cores).

Math (per batch b, channel c):
  feat_ave = mean_{h,w} feat                      # (b, c)
  CMA(feat_ave, deg) -> cma; emb = gamma*cma + deg
  kern = (lrelu(emb @ k_w1.T) @ k_w2.T)           # per-(b,c) 3x3 kernel
  z    = lrelu(depthwise3x3(feat, kern))
  out  = conv_w @ z + conv_b + feat * sigmoid(lrelu(deg@ca_w1.T)@ca_w2.T)

Sharding: data-parallel over batch, 2 batches/core -> 128 partitions=(b,c).

Engine split per group (2 output rows, 512 px):
  - DVE:  3 center taps (kx==1, 4B-aligned windows) as 4x-mode
          tensor_scalar muls + two 2x-mode adds -> acc (bf16).
  - PE:   6 side taps as diagonal-weight bf16 matmuls into PSUM pd, an
          identity matmul accumulating acc into pd (join), the 1x1-conv
          (block-diag conv_w) and a diag(att) residual matmul into PSUM po.
  - Scalar: y = Prelu(pd) in one op; out = po + conv_b via Identity+bias.

Software pipelining: feat lives in four bf16 quarter caches (66 padded rows
each, pw=260: 2 pad cols per side keeps center windows 4B aligned).  A
prologue fills them once.  Each For_i body runs the CMA chain + main loop
on the current caches while textually interleaved refill ops (DMA -> scalar
Copy -> DVE row-sum) rewrite quarter q right after its groups are consumed,
so the next iteration's load hides under this iteration's compute.
"""
import contextlib

import numpy as np

import concourse.bass as bass
import concourse.bacc as bacc
import concourse.tile as tile
import concourse.mybir as mybir
from concourse.masks import make_identity

f32 = mybir.dt.float32
bf16 = mybir.dt.bfloat16
AF = mybir.ActivationFunctionType
OP = mybir.AluOpType

B, C, H, W = 16, 64, 256, 256
NCORES = 8
BPC = B // NCORES          # batches per core
P = BPC * C                # 128 partitions

SIDE = [0, 2, 3, 5, 6, 8]  # taps with kx != 1 (PE diag matmuls)
CENT = [1, 4, 7]           # taps with kx == 1 (DVE mul/add chain)


def build_nc(h=H, w=W, loop_reps=1, prelu_ok=True):
    """Build the per-core SPMD Bass module (shapes [BPC,C,h,w]).

    loop_reps>1 wraps the steady-state body in a hardware For_i loop
    (timing).  prelu_ok=False emits relu+stt instead of AF.Prelu so the
    kernel can run under CoreSim (which lacks Prelu)."""
    pw = w + 4                 # padded row width: 2 pad cols each side
    qr = min(64, h)            # data rows per quarter cache tile
    nq = h // qr
    qrows = qr + 2             # rows per quarter tile (1-row halo each side)
    qn = qrows * pw
    npx = h * w
    n_groups = h // 2
    gpq = qr // 2              # groups per quarter
    lr = 8                     # image rows per load slab
    spq = qr // lr             # slabs per quarter

    nc = bacc.Bacc(trn_type="TRN2")

    feat = nc.dram_tensor("feat", [BPC, C, h, w], f32, kind="ExternalInput")
    deg = nc.dram_tensor("deg", [BPC, C], f32, kind="ExternalInput")
    wq = nc.dram_tensor("wq", [C, C], f32, kind="ExternalInput")
    bq = nc.dram_tensor("bq", [C], f32, kind="ExternalInput")
    wk = nc.dram_tensor("wk", [C, C], f32, kind="ExternalInput")
    bk = nc.dram_tensor("bk", [C], f32, kind="ExternalInput")
    wv = nc.dram_tensor("wv", [C, C], f32, kind="ExternalInput")
    bv = nc.dram_tensor("bv", [C], f32, kind="ExternalInput")
    gamma = nc.dram_tensor("gamma", [1], f32, kind="ExternalInput")
    k_w1 = nc.dram_tensor("k_w1", [C, C], f32, kind="ExternalInput")
    k_w2 = nc.dram_tensor("k_w2", [C * 9, C], f32, kind="ExternalInput")
    conv_w = nc.dram_tensor("conv_w", [C, C], f32, kind="ExternalInput")
    conv_b = nc.dram_tensor("conv_b", [C], f32, kind="ExternalInput")
    ca_w1 = nc.dram_tensor("ca_w1", [C // 8, C], f32, kind="ExternalInput")
    ca_w2 = nc.dram_tensor("ca_w2", [C, C // 8], f32, kind="ExternalInput")
    out = nc.dram_tensor("out", [BPC, C, h, w], f32, kind="ExternalOutput")

    featv = feat[:, :, :, :].rearrange("b c h w -> (b c) (h w)")
    outv = out[:, :, :, :].rearrange("b c h w -> (b c) (h w)")

    with tile.TileContext(nc) as tc, contextlib.ExitStack() as ctx:
        sing = ctx.enter_context(tc.tile_pool(name="sing", bufs=1))
        work = ctx.enter_context(tc.tile_pool(name="work", bufs=3))
        dr = ctx.enter_context(tc.tile_pool(name="dr", bufs=1, space="DRAM"))
        ps_v = ctx.enter_context(tc.tile_pool(name="ps_v", bufs=1, space="PSUM"))
        ps_d = ctx.enter_context(tc.tile_pool(name="ps_d", bufs=3, space="PSUM"))
        ps_o = ctx.enter_context(tc.tile_pool(name="ps_o", bufs=2, space="PSUM"))
        stgp = ctx.enter_context(tc.tile_pool(name="stgp", bufs=2))

        def lrelu_act(out_ap, in_ap, tagname):
            """out = lrelu(in); one scalar op on HW (Prelu honors alpha)."""
            if prelu_ok:
                nc.scalar.activation(out=out_ap, in_=in_ap, func=AF.Prelu,
                                     bias=0.0, scale=1.0, alpha=0.1)
            else:
                tr = work.tile([128] + [d[1] for d in in_ap.ap[1:]], f32,
                               tag=f"lr{tagname}", name=f"lr{tagname}")
                nc.scalar.activation(out=tr[...], in_=in_ap, func=AF.Relu,
                                     bias=0.0, scale=0.9)
                nc.vector.scalar_tensor_tensor(out_ap, in_ap, 0.1, tr[...],
                                               op0=OP.mult, op1=OP.add)

        # ===== persistent tiles (prologue-initialized, live across body) =====
        ident_b = sing.tile([128, 128], bf16)
        make_identity(nc, ident_b[:, :])

        def load_T(src_dram, rows, cols, name):
            t = sing.tile([cols, rows], f32, tag=f"T{name}", name=f"T{name}")
            ap = bass.AP(tensor=src_dram[:, :].tensor, offset=0,
                         ap=[[1, cols], [cols, rows]])
            nc.sync.dma_start(out=t[:, :], in_=ap)
            return t

        def blkdiag(tsb, rows, cols, dtype=f32, name=""):
            blk = sing.tile([128, 128], dtype, tag=f"blk{name}",
                            name=f"blk{name}")
            nc.gpsimd.memset(blk[:, :], 0.0)
            nc.vector.tensor_copy(blk[0:rows, 0:cols], tsb[:, :])
            nc.sync.dma_start(out=blk[64:64 + rows, 64:64 + cols],
                              in_=tsb[:, :])
            return blk

        wqT = load_T(wq, 64, 64, "wq")
        wkT = load_T(wk, 64, 64, "wk")
        wvT = load_T(wv, 64, 64, "wv")
        k_w1T = load_T(k_w1, 64, 64, "kw1")
        conv_wT = load_T(conv_w, 64, 64, "cw")
        ca_w1T = load_T(ca_w1, 8, 64, "ca1")      # [64, 8]
        ca_w2T = load_T(ca_w2, 64, 8, "ca2")      # [8, 64]

        BQ = blkdiag(wqT, 64, 64, name="q")
        BK = blkdiag(wkT, 64, 64, name="k")
        BV = blkdiag(wvT, 64, 64, name="v")
        BW1 = blkdiag(k_w1T, 64, 64, name="w1")
        BA1 = blkdiag(ca_w1T, 64, 8, name="a1")
        BA2 = blkdiag(ca_w2T, 8, 64, name="a2")
        conv_wTb = sing.tile([64, 64], bf16)
        nc.vector.tensor_copy(conv_wTb[:, :], conv_wT[:, :])
        CB = blkdiag(conv_wTb, 64, 64, dtype=bf16, name="cw")

        BK2 = []
        for t in range(9):
            w2t = sing.tile([64, 64], f32, tag=f"w2T{t}", name=f"w2T{t}")
            ap = bass.AP(tensor=k_w2[:, :].tensor, offset=t * 64,
                         ap=[[1, 64], [9 * 64, 64]])
            nc.sync.dma_start(out=w2t[:, :], in_=ap)
            BK2.append(blkdiag(w2t, 64, 64, name=f"k2_{t}"))

        def bcast_c(src, name):
            t = sing.tile([128, 1], f32, tag=f"pc{name}", name=f"pc{name}")
            ap = bass.AP(tensor=src[:].tensor, offset=0,
                         ap=[[0, BPC], [1, C]])
            nc.gpsimd.dma_start(out=t[:, 0:1], in_=ap)
            return t

        bq_pc = bcast_c(bq, "bq")
        bk_pc = bcast_c(bk, "bk")
        bv_pc = bcast_c(bv, "bv")
        conv_b_pc = bcast_c(conv_b, "cb")
        gamma_pc = sing.tile([128, 1], f32)
        nc.gpsimd.dma_start(
            out=gamma_pc[:, 0:1],
            in_=bass.AP(tensor=gamma[:].tensor, offset=0,
                        ap=[[0, 128], [1, 1]]))
        deg_pc = sing.tile([128, 1], f32)
        nc.sync.dma_start(out=deg_pc[:, 0:1],
                          in_=deg[:, :].rearrange("b c -> (b c)")
                          .rearrange("(p one) -> p one", one=1))

        def vec_mm(blk_w, rhs_pc, name):
            p = ps_v.tile([128, 1], f32, tag="vec", name=f"vm{name}")
            nc.tensor.matmul(p[:, 0:1], blk_w[:, :], rhs_pc[:, 0:1],
                             start=True, stop=True)
            return p

        def lrelu_vec(psum_in, name):
            o = sing.tile([128, 1], f32, tag=f"lro{name}", name=f"lro{name}")
            lrelu_act(o[:, 0:1], psum_in[:, 0:1], name)
            return o

        # --------- channel attention (depends only on deg; prologue) --------
        a0 = vec_mm(BA1, deg_pc, "a0")
        t_pr = lrelu_vec(a0, "a0")
        a1 = vec_mm(BA2, t_pr, "a1")
        att_pc = sing.tile([128, 1], f32)
        nc.scalar.activation(out=att_pc[:, 0:1], in_=a1[:, 0:1],
                             func=AF.Sigmoid, bias=0.0, scale=1.0)
        ATTD = sing.tile([128, 128], bf16)
        nc.vector.tensor_scalar_mul(ATTD[:, :], ident_b[:, :], att_pc[:, 0:1])

        kk0 = vec_mm(BK, deg_pc, "kk")
        kk_pi = sing.tile([128, 1], f32)
        nc.scalar.activation(out=kk_pi[:, 0:1], in_=kk0[:, 0:1],
                             func=AF.Identity, bias=bk_pc[:, 0:1], scale=1.0)
        v0 = vec_mm(BV, deg_pc, "v")
        v_pi = sing.tile([128, 1], f32)
        nc.scalar.activation(out=v_pi[:, 0:1], in_=v0[:, 0:1],
                             func=AF.Identity, bias=bv_pc[:, 0:1], scale=1.0)

        def rep64(src_pc, name):
            d = dr.tile([128], f32, tag=f"dr{name}", name=f"dr{name}")
            nc.sync.dma_start(out=d[:], in_=src_pc[:, 0:1])
            rep = sing.tile([128, 64], f32, tag=f"rep{name}",
                            name=f"rep{name}")
            ap = bass.AP(tensor=d[:].tensor, offset=d[:].offset,
                         ap=[[64, BPC], [0, C], [1, C]])
            nc.sync.dma_start(out=rep[:, :], in_=ap)
            return rep

        kk_rep = rep64(kk_pi, "kk")
        v_rep = rep64(v_pi, "v")

        # --------- quarter caches + pad memsets (prologue only) -------------
        quarts = []
        for q in range(nq):
            cq = sing.tile([128, qn], bf16, tag=f"cq{q}", name=f"cq{q}")
            quarts.append(cq)
            cqv = cq[:, :].rearrange("p (r z) -> p r z", z=pw)
            nc.gpsimd.memset(cqv[:, :, 0:2], 0.0)
            nc.gpsimd.memset(cqv[:, :, w + 2:w + 4], 0.0)
            if q == 0:
                nc.gpsimd.memset(cq[:, 0:pw], 0.0)
            if q == nq - 1:
                nc.gpsimd.memset(cq[:, (qrows - 1) * pw:qn], 0.0)
        partials = sing.tile([128, nq * spq], f32)

        def refill_quarter(q):
            """DMA + Copy + row sums for quarter q (8-row slabs + halos)."""
            cq = quarts[q]
            for i in range(spq):
                r_img = q * qr + i * lr
                dst = cq[:, (1 + i * lr) * pw:(1 + (i + 1) * lr) * pw] \
                    .rearrange("p (r z) -> p r z", z=pw)[:, :, 2:2 + w]
                stg = stgp.tile([128, lr * w], f32, tag="stg", name="stg")
                nc.sync.dma_start(out=stg[:, :],
                                  in_=featv[:, r_img * w:(r_img + lr) * w])
                nc.scalar.activation(
                    out=dst, in_=stg[:, :].rearrange("p (r z) -> p r z", z=w),
                    func=AF.Copy, bias=0.0, scale=1.0)
                nc.vector.reduce_sum(partials[:, q * spq + i:q * spq + i + 1],
                                     stg[:, :], axis=mybir.AxisListType.X)
            for (row_img, row_tile) in ((q * qr - 1, 0),
                                        ((q + 1) * qr, qrows - 1)):
                if row_img < 0 or row_img >= h:
                    continue
                hst = stgp.tile([128, w], f32, tag="hst", name="hst")
                nc.sync.dma_start(out=hst[:, :],
                                  in_=featv[:, row_img * w:(row_img + 1) * w])
                nc.scalar.activation(
                    out=cq[:, row_tile * pw + 2:row_tile * pw + 2 + w],
                    in_=hst[:, :], func=AF.Copy, bias=0.0, scale=1.0)

        # prologue load of all quarters
        for q in range(nq):
            refill_quarter(q)

        def win(g, idx):
            """[128, 2, w] window for tap idx at group g."""
            q = g // gpq
            rho = 2 * (g - q * gpq) + 1       # tile-local padded row
            ky, kx = idx // 3, idx % 3
            cap = quarts[q][:, :]
            base = (rho + ky - 1) * pw + 2 + (kx - 1)
            return bass.AP(tensor=cap.tensor, offset=cap.offset + base,
                           ap=[list(cap.ap[0]), [pw, 2], [1, w]])

        def body():
            # ---- CMA + kernel-predictor chain (uses partials from the
            # previous refill wave) ----
            feat_ave = sing.tile([128, 1], f32, tag="fave", name="fave")
            nc.vector.reduce_sum(feat_ave[:, 0:1], partials[:, :],
                                 axis=mybir.AxisListType.X)
            nc.vector.tensor_scalar_mul(feat_ave[:, 0:1], feat_ave[:, 0:1],
                                        1.0 / npx)
            q0 = vec_mm(BQ, feat_ave, "q")
            q_pj = sing.tile([128, 1], f32, tag="qpj", name="qpj")
            nc.scalar.activation(out=q_pj[:, 0:1], in_=q0[:, 0:1],
                                 func=AF.Identity, bias=bq_pc[:, 0:1],
                                 scale=1.0)
            energy = sing.tile([128, C], f32, tag="energy", name="energy")
            nc.vector.tensor_scalar_mul(energy[:, :], kk_rep[:, :],
                                        q_pj[:, 0:1])
            ee = sing.tile([128, C], f32, tag="ee", name="ee")
            nc.scalar.activation(out=ee[:, :], in_=energy[:, :], func=AF.Exp,
                                 bias=0.0, scale=1.0)
            es = sing.tile([128, 1], f32, tag="es", name="es")
            nc.vector.reduce_sum(es[:, 0:1], ee[:, :],
                                 axis=mybir.AxisListType.X)
            erc = sing.tile([128, 1], f32, tag="erc", name="erc")
            nc.vector.reciprocal(erc[:, 0:1], es[:, 0:1])
            attn = sing.tile([128, C], f32, tag="attn", name="attn")
            nc.vector.tensor_scalar_mul(attn[:, :], ee[:, :], erc[:, 0:1])
            prod = sing.tile([128, C], f32, tag="prodt", name="prodt")
            nc.vector.tensor_mul(prod[:, :], attn[:, :], v_rep[:, :])
            cma = sing.tile([128, 1], f32, tag="cma", name="cma")
            nc.vector.reduce_sum(cma[:, 0:1], prod[:, :],
                                 axis=mybir.AxisListType.X)
            emb = sing.tile([128, 1], f32, tag="emb", name="emb")
            nc.vector.scalar_tensor_tensor(emb[:, 0:1], cma[:, 0:1],
                                           gamma_pc[:, 0:1], deg_pc[:, 0:1],
                                           op0=OP.mult, op1=OP.add)
            hid0 = vec_mm(BW1, emb, "hid")
            hid_pc = lrelu_vec(hid0, "hid")
            ktp = ps_v.tile([128, 9], f32, tag="ktap", name="ktp")
            for t in range(9):
                nc.tensor.matmul(ktp[:, t:t + 1], BK2[t][:, :],
                                 hid_pc[:, 0:1], start=True, stop=True)
            k_tap = sing.tile([128, 9], f32, tag="ktapsb", name="k_tap")
            nc.scalar.copy(k_tap[:, :], ktp[:, :])
            diags = {}
            for t in SIDE:
                dg = sing.tile([128, 128], bf16, tag=f"diag{t}",
                               name=f"diag{t}")
                nc.vector.tensor_scalar_mul(dg[:, :], ident_b[:, :],
                                            k_tap[:, t:t + 1])
                diags[t] = dg

            # ---- main loop with interleaved next-wave refills ----
            pair_holder = [None]
            for g in range(n_groups):
                a0t = work.tile([128, 2, w], bf16, tag="a0t", name="a0t")
                nc.vector.tensor_scalar_mul(a0t[:, :, :], win(g, CENT[0]),
                                            k_tap[:, CENT[0]:CENT[0] + 1])
                a1t = work.tile([128, 2, w], bf16, tag="a1t", name="a1t")
                nc.vector.tensor_scalar_mul(a1t[:, :, :], win(g, CENT[1]),
                                            k_tap[:, CENT[1]:CENT[1] + 1])
                a2t = work.tile([128, 2, w], bf16, tag="a2t", name="a2t")
                nc.vector.tensor_scalar_mul(a2t[:, :, :], win(g, CENT[2]),
                                            k_tap[:, CENT[2]:CENT[2] + 1])
                s1 = work.tile([128, 2, w], bf16, tag="s1", name="s1")
                nc.vector.tensor_add(s1[:, :, :], a0t[:, :, :], a1t[:, :, :])
                acc = work.tile([128, 2, w], bf16, tag="acc", name="acc")
                nc.vector.tensor_add(acc[:, :, :], s1[:, :, :], a2t[:, :, :])
                pd = ps_d.tile([128, 2 * w], f32, tag="pd", name="pd")
                for j, idx in enumerate(SIDE):
                    nc.tensor.matmul(pd[:, :], diags[idx][:, :], win(g, idx),
                                     start=(j == 0), stop=False)
                nc.tensor.matmul(pd[:, :], ident_b[:, :],
                                 acc[:, :, :].rearrange("p a b -> p (a b)"),
                                 start=False, stop=True)
                y = work.tile([128, 2 * w], bf16, tag="y", name="y")
                lrelu_act(y[:, :], pd[:, :], "y")
                po = ps_o.tile([128, 2 * w], f32, tag="po", name="po")
                nc.tensor.matmul(po[:, :], CB[:, :], y[:, :],
                                 start=True, stop=False)
                nc.tensor.matmul(po[:, :], ATTD[:, :], win(g, 4),
                                 start=False, stop=True)
                if g % 2 == 0:
                    out_pair = work.tile([128, 4 * w], f32, tag="out_pair",
                                         name="out_pair")
                    pair_holder[0] = out_pair
                half = (g % 2) * 2 * w
                nc.scalar.activation(out=pair_holder[0][:, half:half + 2 * w],
                                     in_=po[:, :], func=AF.Identity,
                                     bias=conv_b_pc[:, 0:1], scale=1.0)
                if g % 2 == 1:
                    nc.sync.dma_start(
                        out=outv[:, (g - 1) * 2 * w:(g + 1) * 2 * w],
                        in_=pair_holder[0][:, :])
                # refill quarter q right after its last group is consumed
                if (g + 1) % gpq == 0:
                    refill_quarter(g // gpq)

        if loop_reps > 1:
            with tc.For_i(0, loop_reps, 1):
                body()
        else:
            body()

    nc.finalize()
    return nc


_NC_CACHE = {}


def _get_nc(h, w):
    if (h, w) not in _NC_CACHE:
        _NC_CACHE[(h, w)] = build_nc(h, w)
    return _NC_CACHE[(h, w)]


def kernel(**inputs):
    from concourse.bass_utils import run_bass_kernel_spmd

    feat = np.ascontiguousarray(inputs["feat"], dtype=np.float32)
    deg = np.ascontiguousarray(inputs["deg"], dtype=np.float32)
    b, c, h, w = feat.shape
    nc = _get_nc(h, w)

    shared = {k: np.ascontiguousarray(np.asarray(v), dtype=np.float32)
              for k, v in inputs.items() if k not in ("feat", "deg")}
    in_maps = []
    for k in range(NCORES):
        m = dict(shared)
        m["feat"] = feat[k * BPC:(k + 1) * BPC]
        m["deg"] = deg[k * BPC:(k + 1) * BPC]
        in_maps.append(m)

    res = run_bass_kernel_spmd(nc, in_maps, core_ids=list(range(NCORES)))
    return np.concatenate([r["out"] for r in res.results], axis=0)

